# revision 14
# baseline (speedup 1.0000x reference)
"""Trainium2 Bass kernel for the LeViT-style attention block.

Contract: kernel(**inputs) takes the FULL unsharded inputs (numpy) and
returns the FULL [128, 196, 576] float32 output. Internally shards the
batch dim across 8 NeuronCores (16 batches per core) and runs a single
SPMD Bass/Tile program via run_bass_kernel_spmd.

Math (per batch b):
  xn   = LayerNorm(x[b]) * g + beta                     [196, 576]
  qkv  = xn @ qkv_w.T + qkv_b      -> q,k,v per head
  S_h  = (q_h * kd^-0.5) @ k_h.T + bias_h               [196, 196]
  P_h  = softmax(S_h, axis=-1)
  O_h  = P_h @ v_h                                      [196, 128]
  out  = concat_h(O_h) @ proj_w.T + proj_b              [196, 576]

Implementation notes (v2 — optimized against the TimelineSim cost model,
which charges matmuls out_free_size * cycles_per_row with bf16 = 1
cycle/row at ANY width, f32r = 1 only at >=256-wide, fp32 = 4):
  - LN affine (g, beta) and the qk scale are folded into the QKV weights
    host-side; the device LN computes (x - mu) * rsqrt(var+eps) only and
    emits xn in bf16.
  - The bf16 datapath covers LN output -> transposes -> QKV/V GEMMs ->
    scores -> exp -> PV. Scores and PV run at 196-wide (valid columns
    only) since bf16 has no width penalty. proj stays f32r.
  - The relative-position bias is folded in MULTIPLICATIVELY after exp:
    E = exp(S) * EB with EB = exp(bias) precomputed host-side (bf16,
    [key, query] layout). This removes the per-head PSUM bias-seed
    matmuls entirely; the multiply runs on DVE in 2x bf16 mode.
  - Q.T/K.T are produced in 10 groups of <=128 rows (4 heads each, last
    group 2) so per-head slices sit at partition offsets {0,32,64,96}
    with matching Q/K bases.
  - softmax: exp without max subtraction (scores are O(6); fp32-safe);
    denominator = ones-column matmul over keys on PE; reciprocal on DVE;
    broadcast across partitions via gpsimd partition_broadcast (Pool is
    otherwise idle).
  - Per-head O.T stays resident in SBUF (no DRAM round-trip); proj
    accumulates all 18 heads into 5 resident PSUM banks; the result is
    transposed back to token layout (bf16) and stored to a bf16 output.
"""

import os

os.environ.setdefault("MYCRO_LOCAL_CACHE", "1")

from contextlib import ExitStack

import numpy as np
import ml_dtypes

import concourse.bass as bass
import concourse.mybir as mybir
import concourse.tile as tile
from concourse import library_config, masks
from concourse.bass_utils import run_bass_kernel_spmd

# Problem shape (hardcoded per contest contract).
B, N, C = 128, 196, 576
H, KD, DV = 18, 32, 128
DH = H * DV            # 2304
LN_EPS = 1e-5
SCALE = KD ** -0.5
NCORES = 8
BPC = B // NCORES      # 16 batches per core
SB = 2                 # batches per "superbatch" iteration
NSB = BPC // SB        # 8
W = SB * N             # 392: packed two-batch free dim

FP32 = mybir.dt.float32
F32R = mybir.dt.float32r
BF16 = mybir.dt.bfloat16

# token-dim chunks (196 = 128 + 68)
TOK_CHUNKS = [(0, 128), (128, 68)]
# C-dim chunks (576 = 4*128 + 64)
C_CHUNKS = [(i * 128, min(128, C - i * 128)) for i in range((C + 127) // 128)]
NCC = len(C_CHUNKS)    # 5 contraction chunks / proj out-chunks
# Q.T/K.T GEMMs run in 9 chunks of <=128 PSUM rows (minimal matmul count);
# the PSUM->SBUF copies split each chunk into two 64-row (2-head) tiles so
# every copy and per-head slice is quadrant-legal (base {0,32,64}, and a
# base-32 access may cover at most 32 partitions).
NQT = 18               # two-head Q.T/K.T tiles (0..8 = Q, 9..17 = K)
GROWS = 2 * KD         # 64
# V free-dim chunks of 512 = 4 heads (2304 = 4*512 + 256); head-group g
# covers heads 4g..4g+3 (last group: 16,17).
V_CHUNKS = [(i * 512, min(512, DH - i * 512)) for i in range((DH + 511) // 512)]


def _split_multiwaits(nc):
    """This container's walrus rejects >1 sync-wait per instruction
    (TPB EVENTS struct has a single wait slot). Split extras into
    preceding same-engine NOPs — semantically identical."""
    for f in nc.m.functions:
        for blk in f.blocks:
            newlist = []
            changed = False
            for inst in blk.instructions:
                si = inst.sync_info
                waits = list(si.on_wait) if si is not None else []
                if len(waits) > 1:
                    changed = True
                    for j, w in enumerate(waits[:-1]):
                        nop = mybir.InstNoOp(name=f"{inst.name}_sw{j}", ins=[], outs=[])
                        nop.engine = inst.engine
                        nop.sync_info = mybir.SyncInfo(on_wait=[w], on_update=[])
                        newlist.append(nop)
                    inst.sync_info = mybir.SyncInfo(
                        on_wait=[waits[-1]], on_update=list(si.on_update)
                    )
                newlist.append(inst)
            if changed:
                blk.instructions = newlist


def _emit(ctx: ExitStack, tc: tile.TileContext, aps: dict, has_vbias: bool):
    nc = tc.nc
    x_d = aps["x"]          # [BPC, 196, 576] f32
    out_d = aps["out"]      # [BPC, 196, 576] bf16
    wqk_d = aps["wqkt"]     # [576, 1152] bf16 (cols 0:576 Wq.T, 576:1152 Wk.T)
    wv_d = aps["wvt"]       # [576, 2304] bf16 (Wv.T, head-major columns)
    pw_d = aps["pwt"]       # [2304, 576] f32r (proj_w.T, head-major rows)
    eb_d = aps["ebT"]       # [18, 128, 392] bf16 exp(bias) [key-chunked, query]
    vb_d = aps.get("vb")    # [128, 2304] f32 (replicated v bias) — optional

    cpool = ctx.enter_context(tc.tile_pool(name="consts", bufs=1))
    xpool = ctx.enter_context(tc.tile_pool(name="x", bufs=4))
    stat = ctx.enter_context(tc.tile_pool(name="stat", bufs=2))
    xnbpool = ctx.enter_context(tc.tile_pool(name="xnb", bufs=4))
    xnt_pool = ctx.enter_context(tc.tile_pool(name="xnt", bufs=2))
    qkt_pool = ctx.enter_context(tc.tile_pool(name="qkt", bufs=1))
    vpool = ctx.enter_context(tc.tile_pool(name="v", bufs=2))
    epool = ctx.enter_context(tc.tile_pool(name="e", bufs=3))
    empool = ctx.enter_context(tc.tile_pool(name="em", bufs=6))
    rcpool = ctx.enter_context(tc.tile_pool(name="rc", bufs=3))
    onpool = ctx.enter_context(tc.tile_pool(name="on", bufs=1))
    rbpool = ctx.enter_context(tc.tile_pool(name="rb", bufs=3))
    ftpool = ctx.enter_context(tc.tile_pool(name="ft", bufs=1))
    fpool = ctx.enter_context(tc.tile_pool(name="f", bufs=2))
    ps = ctx.enter_context(tc.tile_pool(name="ps", bufs=8, space="PSUM"))

    # ---- small on-chip constants first ----
    ident_f = cpool.tile([128, 128], FP32, tag="ident_f", name="ident_f")
    masks.make_identity(nc, ident_f[:])
    ident_r = cpool.tile([128, 128], F32R, tag="ident_r", name="ident_r")
    nc.vector.tensor_copy(ident_r[:], ident_f[:])
    ones_fb = cpool.tile([128, 1], FP32, tag="ones_fb", name="ones_fb")
    nc.gpsimd.memset(ones_fb[:], 1.0)
    onescol = cpool.tile([128, 1], BF16, tag="onescol", name="onescol")
    nc.vector.tensor_copy(onescol[:], ones_fb[:])
    epsb = cpool.tile([128, 1], FP32, tag="epsb", name="epsb")
    nc.gpsimd.memset(epsb[:], LN_EPS)
    onesrow_f = cpool.tile([1, 128], FP32, tag="onesrow_f", name="onesrow_f")
    nc.gpsimd.memset(onesrow_f[:], 1.0)
    onesrow = cpool.tile([1, 128], F32R, tag="onesrow", name="onesrow")
    nc.vector.tensor_copy(onesrow[:], onesrow_f[:])

    # ---- constants (loaded once, resident), first-use order, on the
    # Pool/SWDGE queue so x/out streaming (HWDGE) is unaffected ----
    wqk = []
    for ci, (c0, cs) in enumerate(C_CHUNKS):
        t = cpool.tile([128, 2 * C], BF16, tag=f"wqk{ci}", name=f"wqk{ci}")
        nc.sync.dma_start(t[:cs, :], wqk_d[c0 : c0 + cs, :])
        wqk.append(t)
    eb = []
    for h in range(H):
        eb.append(cpool.tile([128, W], BF16, tag=f"eb{h}", name=f"eb{h}"))
    wv = {}
    # interleave V-weight column groups with the EB heads they feed, in
    # first-use order, so the first superbatch starts as early as possible;
    # per-(ci,group) tiles so deps resolve as soon as each group lands
    for g, (n0, ns) in enumerate(V_CHUNKS):
        for ci, (c0, cs) in enumerate(C_CHUNKS):
            t = cpool.tile([128, 512], BF16, tag=f"wv{ci}g{g}", name=f"wv{ci}g{g}")
            nc.sync.dma_start(t[:cs, :ns], wv_d[c0 : c0 + cs, n0 : n0 + ns])
            wv[(ci, g)] = t
        for h in range(4 * g, min(4 * g + 4, H)):
            nc.sync.dma_start(eb[h][:, :], eb_d[h])
    vb = None
    if has_vbias:
        vb = cpool.tile([128, DH], FP32, tag="vb", name="vb")
        nc.sync.dma_start(vb[:], vb_d[:])
    pw = []
    for h in range(H):
        t = cpool.tile([128, C], F32R, tag=f"pw{h}", name=f"pw{h}")
        nc.sync.dma_start(t[:], pw_d[h * DV : (h + 1) * DV, :])
        pw.append(t)

    inv_c = 1.0 / C

    def em_pair(em, q_off, tj, ts_):
        """Both batches' E for key-chunk tj as a [ts_, 2, 196] AP."""
        return em[:ts_, 0 : 4 * N].rearrange(
            "p (q c n) -> p c q n", q=2, c=2, n=N
        )[:, tj]

    lnstate = {}
    xstate = {}

    def ln_load(sbx, q, tj):
        b = sbx * SB + q
        t0, ts_ = TOK_CHUNKS[tj]
        xt = xpool.tile([128, C], FP32, tag="xb", name=f"xb{q}{tj}_{sbx}")
        nc.scalar.dma_start(xt[:ts_, :], x_d[b, t0 : t0 + ts_, :])
        xstate[(sbx, q, tj)] = xt

    def ln_chunk(sbx, q, tj):
        """LayerNorm one loaded (batch, token-chunk) of x -> bf16 xn."""
        _CUR[0] = f"ln{q}{tj}_{sbx}" 
        t0, ts_ = TOK_CHUNKS[tj]
        xt = xstate.pop((sbx, q, tj))
        negmu = stat.tile([128, 1], FP32, tag="negmu", name=f"nm{q}{tj}_{sbx}")
        nc.vector.tensor_reduce(
            negmu[:ts_], xt[:ts_, :], axis=mybir.AxisListType.X,
            op=mybir.AluOpType.add, negate=True,
        )
        nc.vector.tensor_scalar_mul(negmu[:ts_], negmu[:ts_], inv_c)
        # squared deviations in two halves (PSUM scratch; only the per-row
        # accumulators matter, the second half overwrites the first)
        sqp = ps.tile([128, 512], FP32, tag="ps", name=f"sqp{q}{tj}_{sbx}")
        ha = stat.tile([128, 1], FP32, tag="ha", name=f"ha{q}{tj}_{sbx}")
        hb = stat.tile([128, 1], FP32, tag="hb", name=f"hb{q}{tj}_{sbx}")
        nc.scalar.activation(
            sqp[:ts_, 0:288], xt[:ts_, 0:288],
            mybir.ActivationFunctionType.Square, bias=negmu[:ts_], accum_out=ha[:ts_],
        )
        nc.scalar.activation(
            sqp[:ts_, 0:288], xt[:ts_, 288:576],
            mybir.ActivationFunctionType.Square, bias=negmu[:ts_], accum_out=hb[:ts_],
        )
        ssq = stat.tile([128, 1], FP32, tag="ssq", name=f"ssq{q}{tj}_{sbx}")
        nc.vector.tensor_add(ssq[:ts_], ha[:ts_], hb[:ts_])
        std = stat.tile([128, 1], FP32, tag="std", name=f"std{q}{tj}_{sbx}")
        nc.scalar.activation(
            std[:ts_], ssq[:ts_], mybir.ActivationFunctionType.Sqrt,
            bias=epsb[:ts_], scale=inv_c,
        )
        r = stat.tile([128, 1], FP32, tag="r", name=f"r{q}{tj}_{sbx}")
        nc.vector.reciprocal(r[:ts_], std[:ts_])
        negmur = stat.tile([128, 1], FP32, tag="negmur", name=f"nr{q}{tj}_{sbx}")
        nc.vector.tensor_mul(negmur[:ts_], negmu[:ts_], r[:ts_])
        # xn = (x - mu) * r, emitted bf16
        xb = xnbpool.tile([128, C], F32R, tag="xnb", name=f"xnb{q}{tj}_{sbx}")
        nc.scalar.activation(
            xb[:ts_, :], xt[:ts_, :], mybir.ActivationFunctionType.Identity,
            bias=negmur[:ts_], scale=r[:ts_],
        )
        lnstate[(sbx, q, tj)] = xb

    pending_finals = []

    _CUR[0] = "ln0"
    for q in range(SB):
        for tj in range(len(TOK_CHUNKS)):
            ln_load(0, q, tj)
    for q in range(SB):
        for tj in range(len(TOK_CHUNKS)):
            ln_chunk(0, q, tj)

    for sb in range(NSB):
        _CUR[0] = f"transp_{sb}"
        # ---- transpose xn -> xnT (channel-major bf16, both batches) ----
        xnt = [
            xnt_pool.tile([128, W], BF16, tag=f"xnt{ci}", name=f"xnt{ci}_{sb}")
            for ci in range(NCC)
        ]
        for q in range(SB):
            for tj, (t0, ts_) in enumerate(TOK_CHUNKS):
                xb = lnstate.pop((sb, q, tj))
                for ci, (c0, cs) in enumerate(C_CHUNKS):
                    pt = ps.tile([128, 512], F32R, tag="ps", name=f"pst{q}{tj}{ci}_{sb}")
                    nc.tensor.transpose(
                        pt[:cs, :ts_], xb[:ts_, c0 : c0 + cs], ident_r[:ts_, :ts_]
                    )
                    col = q * N + t0
                    if ci % 2 == 0:
                        nc.scalar.copy(xnt[ci][:cs, col : col + ts_], pt[:cs, :ts_])
                    else:
                        nc.vector.tensor_copy(xnt[ci][:cs, col : col + ts_], pt[:cs, :ts_])

        if sb + 1 < NSB:
            # next superbatch's x loads: slots just freed by the transposes
            for q in range(SB):
                for tj in range(len(TOK_CHUNKS)):
                    ln_load(sb + 1, q, tj)

        # ---- Q.T / K.T GEMMs: 9 chunks of <=128 rows (minimal matmul
        # count); copies split each chunk into two 64-row (2-head) tiles ----
        qkt = [
            qkt_pool.tile([GROWS, W], BF16, tag=f"qkt{t}", name=f"qkt{t}_{sb}")
            for t in range(NQT)
        ]
        ncopy = 0
        for j in range(9):
            _CUR[0] = f"qkv{j}_{sb}"
            r0 = 128 * j              # global row (Q rows 0:576, K rows 576:1152)
            gr = min(128, 2 * C - r0)
            pq = ps.tile([128, 512], FP32, tag="ps", name=f"psqk{j}_{sb}")
            for ci, (c0, cs) in enumerate(C_CHUNKS):
                nc.tensor.matmul(
                    pq[:gr, :W], wqk[ci][:cs, r0 : r0 + gr],
                    xnt[ci][:cs, :W],
                    start=(ci == 0), stop=(ci == NCC - 1),
                )
            for half in range(gr // GROWS):
                t_idx = 2 * j + half
                srcap = pq[half * GROWS : (half + 1) * GROWS, :W]
                if ncopy % 2 == 0:
                    nc.scalar.copy(qkt[t_idx][:, :], srcap)
                else:
                    nc.vector.tensor_copy(qkt[t_idx][:, :], srcap)
                ncopy += 1

        # ---- attention: software-pipelined head loop (skew 2) ----
        vgroups = {}
        emstate = {}
        bstate = {}
        onorm = {}

        def stage_v(g, sb=sb, xnt=xnt, vgroups=vgroups):
            _CUR[0] = f"v{g}_{sb}"
            n0, ns = V_CHUNKS[g]
            vt = {}
            for q in range(SB):
                for tj, (t0, ts_) in enumerate(TOK_CHUNKS):
                    v = vpool.tile([128, 512], BF16, tag=f"v{q}{tj}",
                                   name=f"v{q}{tj}g{g}_{sb}")
                    pv = ps.tile([128, 512], FP32, tag="ps", name=f"psv{q}{tj}{g}_{sb}")
                    for ci, (c0, cs) in enumerate(C_CHUNKS):
                        nc.tensor.matmul(
                            pv[:ts_, :ns], xnt[ci][:cs, q * N + t0 : q * N + t0 + ts_],
                            wv[(ci, g)][:cs, :ns],
                            start=(ci == 0), stop=(ci == NCC - 1),
                        )
                    if has_vbias:
                        nc.vector.tensor_add(
                            v[:ts_, :ns], pv[:ts_, :ns], vb[:ts_, n0 : n0 + ns]
                        )
                    elif (q + tj) % 2 == 0:
                        nc.scalar.copy(v[:ts_, :ns], pv[:ts_, :ns])
                    else:
                        nc.vector.tensor_copy(v[:ts_, :ns], pv[:ts_, :ns])
                    vt[(q, tj)] = v
            vgroups[g] = vt

        def stage_a(h, sb=sb, qkt=qkt, emstate=emstate, vgroups=vgroups,
                    stage_v=stage_v):
            # scores + exp + EB-multiply for head h. S.T in [key, query]
            # layout, valid 196 columns only (bf16 matmul has no width
            # penalty). Per batch: one PSUM bank holds both key chunks
            # ([128,196] at cols 0:196, [68,196] at cols 196:392).
            if h // 4 not in vgroups:
                stage_v(h // 4)
            _CUR[0] = f"sa{h}_{sb}"
            qt = qkt[h // 2]
            kt = qkt[9 + h // 2]
            r0 = 32 * (h % 2)
            e = epool.tile([128, 2 * W], BF16, tag="e", name=f"e{h}_{sb}")
            em = empool.tile([128, 2 * W], BF16, tag="em", name=f"em{h}_{sb}")
            for q in range(SB):
                s = ps.tile([128, 512], FP32, tag="ps", name=f"st{q}h{h}_{sb}")
                for tj, (t0, ts_) in enumerate(TOK_CHUNKS):
                    nc.tensor.matmul(
                        s[:ts_, tj * N : (tj + 1) * N],
                        kt[r0 : r0 + 32, q * N + t0 : q * N + t0 + ts_],
                        qt[r0 : r0 + 32, q * N : q * N + N],
                        start=True, stop=True, skip_group_check=True,
                    )
                # blocks [A0|A1|B0|B1] at 196 stride; one exp per batch
                nc.scalar.activation(
                    e[:, q * W : (q + 1) * W], s[:, 0:W],
                    mybir.ActivationFunctionType.Exp,
                )
                nc.vector.tensor_mul(
                    em[:, q * W : (q + 1) * W], e[:, q * W : (q + 1) * W],
                    eb[h][:, :],
                )
            emstate[h] = em

        def stage_b1(h, sb=sb, emstate=emstate, bstate=bstate, vgroups=vgroups):
            # denominator + reciprocal, and PV over unnormalized E
            _CUR[0] = f"sb1_{h}_{sb}"
            em = emstate.pop(h)
            g = h // 4
            vt = vgroups[g]
            n0, ns = V_CHUNKS[g]
            hcol = h * DV - n0
            dn = ps.tile([1, W], FP32, tag="ps", name=f"dn{h}_{sb}")
            for tj, (t0, ts_) in enumerate(TOK_CHUNKS):
                nc.tensor.matmul(
                    dn[:1, :W], onescol[:ts_, :], em_pair(em, 0, tj, ts_),
                    start=(tj == 0), stop=(tj == 1),
                )
            rc = rcpool.tile([1, W], F32R, tag="rc", name=f"rc{h}_{sb}")
            nc.vector.reciprocal(rc[:], dn[:1, :W])
            ou = ps.tile([128, W], FP32, tag="ps", name=f"ou{h}_{sb}")
            for q in range(SB):
                for tj, (t0, ts_) in enumerate(TOK_CHUNKS):
                    nc.tensor.matmul(
                        ou[:DV, q * N : (q + 1) * N],
                        vt[(q, tj)][:ts_, hcol : hcol + DV],
                        em[:ts_, (2 * q + tj) * N : (2 * q + tj + 1) * N],
                        start=(tj == 0), stop=(tj == 1),
                    )
            bstate[h] = (rc, ou)

        def stage_b2(h, sb=sb, bstate=bstate, onorm=onorm):
            # broadcast reciprocal across partitions (PE rank-1), normalize
            # O.T with both operands read from PSUM
            _CUR[0] = f"sb2_{h}_{sb}"
            rc, ou = bstate.pop(h)
            bc = ps.tile([128, W], FP32, tag="ps", name=f"bc{h}_{sb}")
            nc.tensor.matmul(
                bc[:, :W], onesrow[:1, :], rc[:1, :W], start=True, stop=True
            )
            rb = rbpool.tile([128, W], FP32, tag="rb", name=f"rb{h}_{sb}")
            if h % 2 == 0:
                nc.scalar.copy(rb[:, :], bc[:, :W])
            else:
                nc.vector.tensor_copy(rb[:, :], bc[:, :W])
            on = onpool.tile([128, W], F32R, tag=f"on{h}", name=f"on{h}_{sb}")
            nc.vector.tensor_mul(on[:DV, :], ou[:DV, :W], rb[:DV, :])
            onorm[h] = on

        SKEW = 4
        for hh in range(SKEW):
            stage_a(hh)
        stage_b1(0)
        for h in range(H - 2):
            if pending_finals and h in (0, 1, 2, 4):
                pending_finals.pop(0)()
            if h + SKEW < H:
                stage_a(h + SKEW)
            if h + 1 < H - 2:
                stage_b1(h + 1)
            stage_b2(h)
            if h in (3, 7, 11, 15) and sb + 1 < NSB:
                # hoist next superbatch's LayerNorm, one chunk at a time, so
                # its DVE/ACT work spreads under this superbatch's attention
                k = (3, 7, 11, 15).index(h)
                ln_chunk(sb + 1, k // 2, k % 2)

        # ---- proj: accumulate heads from resident O.T. Heads 16/17's
        # denominator/PV are deferred until proj h0..13 has dispatched, so
        # their wait-queue parking doesn't stall the PE sequencer while
        # their exp/EB-mult chains drain on ACT/DVE. ----
        _CUR[0] = f"proj_{sb}"
        ft = ftpool.tile([128, NCC * W], F32R, tag="ft", name=f"ft{sb}")
        pp = {
            m: ps.tile([128, W], FP32, tag="ps", name=f"pp{m}_{sb}")
            for m in range(NCC)
        }

        def proj_head(h):
            for m in range(NCC):
                c0, mc = C_CHUNKS[m]
                nc.tensor.matmul(
                    pp[m][:mc, :W], pw[h][:, c0 : c0 + mc], onorm[h][:DV, :W],
                    start=(h == 0), stop=(h == H - 1),
                )

        for h in range(14):
            proj_head(h)
        for h in (16, 17):
            stage_b1(h)
            stage_b2(h)
        for h in range(14, H):
            proj_head(h)
        _CUR[0] = f"proj_{sb}"
        for m in range(NCC):
            c0, mc = C_CHUNKS[m]
            if m % 2 == 0:
                nc.scalar.copy(ft[:mc, m * W : m * W + W], pp[m][:mc, :W])
            else:
                nc.vector.tensor_copy(ft[:mc, m * W : m * W + W], pp[m][:mc, :W])

        # ---- transpose back to token layout (bf16) and store: deferred
        # into the next superbatch's head loop so the copies hide under
        # attention (flushed immediately on the last superbatch) ----
        def make_final(q, tj, ft=ft, sb=sb):
            def emit():
                _CUR[0] = f"final{q}{tj}_{sb}"
                b = sb * SB + q
                t0, ts_ = TOK_CHUNKS[tj]
                f = fpool.tile([128, C], BF16, tag="f", name=f"f{q}{tj}_{sb}")
                for m, (c0, mc) in enumerate(C_CHUNKS):
                    pt = ps.tile([128, 512], F32R, tag="ps", name=f"psf{m}{q}{tj}_{sb}")
                    src0 = m * W + q * N + t0
                    nc.tensor.transpose(
                        pt[:ts_, :mc], ft[:mc, src0 : src0 + ts_], ident_r[:mc, :mc]
                    )
                    if m % 2 == 0:
                        nc.scalar.copy(f[:ts_, c0 : c0 + mc], pt[:ts_, :mc])
                    else:
                        nc.vector.tensor_copy(f[:ts_, c0 : c0 + mc], pt[:ts_, :mc])
                nc.sync.dma_start(out_d[b, t0 : t0 + ts_, :], f[:ts_, :])
            return emit

        chunks = [make_final(q, tj) for q in range(SB) for tj in range(2)]
        if sb + 1 < NSB:
            pending_finals.extend(chunks)
        else:
            for c in chunks:
                c()


def _build(has_vbias: bool):
    nc = bass.Bass(
        "TRN2", target_bir_lowering=False, debug=False,
        enable_asserts=False, num_devices=NCORES,
    )
    aps = {}
    aps["x"] = nc.dram_tensor("x", [BPC, N, C], FP32, kind="ExternalInput").ap()
    aps["wqkt"] = nc.dram_tensor("wqkt", [C, 2 * C], BF16, kind="ExternalInput").ap()
    aps["wvt"] = nc.dram_tensor("wvt", [C, DH], BF16, kind="ExternalInput").ap()
    aps["pwt"] = nc.dram_tensor("pwt", [DH, C], F32R, kind="ExternalInput").ap()
    aps["ebT"] = nc.dram_tensor("ebT", [H, 128, W], BF16, kind="ExternalInput").ap()
    if has_vbias:
        aps["vb"] = nc.dram_tensor("vb", [128, DH], FP32, kind="ExternalInput").ap()
    aps["out"] = nc.dram_tensor("out", [BPC, N, C], BF16, kind="ExternalOutput").ap()

    _hook_labels(nc)
    with tile.TileContext(nc) as tc, ExitStack() as ctx:
        with nc.allow_low_precision(reason="bf16/f32r matmul pipeline"):
            _emit(ctx, tc, aps, has_vbias)
    _split_multiwaits(nc)
    return nc


_BUILD_CACHE: dict = {}
_LABELS: dict = {}
_CUR = ["init"]


def _hook_labels(nc):
    orig = nc.get_next_instruction_name
    def wrapped():
        name = orig()
        _LABELS[name] = _CUR[0]
        return name
    nc.get_next_instruction_name = wrapped
    orig2 = nc.next_id
    def wrapped2():
        i = orig2()
        _LABELS[f"I-{i}"] = _CUR[0]
        return i
    nc.next_id = wrapped2


def _prep_host(x, ln_g, ln_b, qkv_w, qkv_b, proj_w, proj_b, attn_biases, bias_idxs):
    """Permute/fold weights host-side. Returns (in_map_consts, has_vbias)."""
    f32 = np.float32
    qkv_w = np.asarray(qkv_w, f32)
    qkv_b = np.asarray(qkv_b, f32)
    ln_g = np.asarray(ln_g, f32)
    ln_b = np.asarray(ln_b, f32)
    proj_w = np.asarray(proj_w, f32)
    proj_b = np.asarray(proj_b, f32)
    attn_biases = np.asarray(attn_biases, f32)
    bias_idxs = np.asarray(bias_idxs)

    per = 2 * KD + DV  # 192 rows per head in qkv_w
    wq = np.concatenate([qkv_w[h * per : h * per + KD] for h in range(H)], 0)
    wk = np.concatenate([qkv_w[h * per + KD : h * per + 2 * KD] for h in range(H)], 0)
    wv = np.concatenate([qkv_w[h * per + 2 * KD : (h + 1) * per] for h in range(H)], 0)
    bq = np.concatenate([qkv_b[h * per : h * per + KD] for h in range(H)], 0)
    bk = np.concatenate([qkv_b[h * per + KD : h * per + 2 * KD] for h in range(H)], 0)
    bv = np.concatenate([qkv_b[h * per + 2 * KD : (h + 1) * per] for h in range(H)], 0)

    # fold LN affine: xn = xn0 * g + beta  =>  W_eff = W*g, b_eff = W@beta + b
    wq_eff = (wq * ln_g[None, :] * SCALE).astype(f32)
    wk_eff = (wk * ln_g[None, :]).astype(f32)
    wv_eff = (wv * ln_g[None, :]).astype(f32)
    bq_eff = ((wq @ ln_b + bq) * SCALE).astype(f32)
    bk_eff = (wk @ ln_b + bk).astype(f32)
    bv_eff = (wv @ ln_b + bv).astype(f32)
    assert not np.any(bq_eff) and not np.any(bk_eff), (
        "nonzero q/k bias not supported in v2 kernel"
    )

    wqkt = np.concatenate([wq_eff.T, wk_eff.T], axis=1)         # [576, 1152]
    wqkt = wqkt.astype(ml_dtypes.bfloat16).copy()
    wvt = wv_eff.T.astype(ml_dtypes.bfloat16).copy()            # [576, 2304]
    pwt = proj_w.T.copy()                                       # [2304, 576]
    assert not np.any(proj_b), "nonzero proj bias not supported in v2 kernel"

    # per-head exp(bias), gathered, transposed to [key, query], and packed
    # as [128 partitions, 392]: cols 0:196 keys 0:128, cols 196:392 keys
    # 128:196 (rows 68:128 zero/unused)
    biasT = attn_biases[:, bias_idxs.T]                         # [H, Nk, Nq]
    ebT = np.zeros((H, 128, W), dtype=np.float32)
    ebT[:, :, 0:N] = np.exp(biasT[:, 0:128, :])
    ebT[:, 0:68, N : 2 * N] = np.exp(biasT[:, 128:196, :])
    ebT = ebT.astype(ml_dtypes.bfloat16)

    has_vbias = bool(np.any(bv_eff != 0.0))
    consts = {
        "wqkt": wqkt, "wvt": wvt, "pwt": pwt,
        "ebT": np.ascontiguousarray(ebT),
    }
    if has_vbias:
        consts["vb"] = np.broadcast_to(bv_eff[None, :], (128, DH)).copy()
    return consts, has_vbias


def kernel(**inputs) -> np.ndarray:
    x = np.asarray(inputs["x"], np.float32)
    consts, has_vbias = _prep_host(
        x, inputs["ln_g"], inputs["ln_b"], inputs["qkv_w"], inputs["qkv_b"],
        inputs["proj_w"], inputs["proj_b"], inputs["attn_biases"],
        inputs["bias_idxs"],
    )
    key = has_vbias
    if key not in _BUILD_CACHE:
        _BUILD_CACHE[key] = _build(has_vbias)
    nc = _BUILD_CACHE[key]

    in_maps = []
    for c in range(NCORES):
        m = {"x": np.ascontiguousarray(x[c * BPC : (c + 1) * BPC])}
        m.update(consts)
        in_maps.append(m)
    res = run_bass_kernel_spmd(nc, in_maps, list(range(NCORES)))
    out = np.concatenate([res.results[c]["out"] for c in range(NCORES)], axis=0)
    return out.astype(np.float32)
